# revision 8
# baseline (speedup 1.0000x reference)
"""Trainium2 Bass kernel for nn_EvolvedNetwork_90117003804689 (dense_mlp).

MLP: x[131072,784] -> 50 -> 30 -> 10 -> 10, with per-neuron activation
select (relu/tanh/sigmoid by act code) after each of the 3 hidden layers.

Strategy (pure data parallel over 8 cores, 16384 rows each):
 - Activations are kept TRANSPOSED on chip: hT [neurons, batch], batch on
   the free dim, so layer matmuls are W.T @ hT with tiny stationary weights.
 - x is DMA-loaded with an f32->bf16 cast (SWDGE), transposed 128x112 at a
   time on the PE (identity transpose) into PSUM, copied to SBUF (DVE/ACT
   alternating) to form the K-major rhs for layer 1.
 - Per-neuron activation select is algebraic, no masks/blends at runtime:
     h = relu_ch + q*tanh_ch + const,  with
     relu channel  r = relu(z + b)                 (DVE tensor_scalar add+max)
     tanh channel  t = tanh(s*(z + b)), s=1-0.5*m2 (ACT, per-partition
                                                    scale/bias)
   sigmoid(v) = 0.5 + 0.5*tanh(v/2) folds into s, q=m1+0.5*m2, and the +0.5
   constant folds into the next layer's bias. The channel masks m0/q fold
   into the next layer's (host-prepared) weights, which take the doubled
   [r; t] stack as input. Each z is produced in doubled form (relu copy at
   partitions 0..d, tanh copy at 64..64+d, zero weight columns elsewhere)
   because engine APs need 32-aligned partition bases and cannot shift
   partitions; the zero columns make PE write exact 0s to gap partitions,
   so downstream reads stay clean.
 - Output is produced as oT [10, 16384] per core; the host transposes and
   concatenates shards.
"""
import os
from contextlib import ExitStack

import numpy as np
import ml_dtypes

from concourse import bacc, mybir
import concourse.tile as tile
from concourse.bass_utils import run_bass_kernel_spmd
from concourse.masks import make_identity

BF16 = ml_dtypes.bfloat16
P = 128
NCORES = 8
B_TOTAL = 131072
B_CORE = B_TOTAL // NCORES  # 16384
D = 784
NCH = 7  # K chunks of 112 for layer 1
CK = 112
D1, D2, D3, DO = 50, 30, 10, 10
BLK = 1024  # rows per block
NBLK = B_CORE // BLK  # 16

_prog_cache = {}


def _build(nblk: int, iters: int = 1):
    nc = bacc.Bacc("TRN2", target_bir_lowering=False)
    f32 = mybir.dt.float32
    bf16 = mybir.dt.bfloat16

    x = nc.dram_tensor("x", [nblk * BLK, D], f32, kind="ExternalInput")
    w1 = nc.dram_tensor("w1", [CK, NCH * P], bf16, kind="ExternalInput")
    w2 = nc.dram_tensor("w2", [P, P], bf16, kind="ExternalInput")
    w3 = nc.dram_tensor("w3", [P, P], bf16, kind="ExternalInput")
    wo = nc.dram_tensor("wo", [P, DO], bf16, kind="ExternalInput")
    scal = nc.dram_tensor("scal", [P, 10], f32, kind="ExternalInput")
    out = nc.dram_tensor("out", [DO, nblk * BLK], f32, kind="ExternalOutput")

    with ExitStack() as ctx:
        tc = ctx.enter_context(tile.TileContext(nc))
        consts = ctx.enter_context(tc.tile_pool(name="consts", bufs=1))
        xpool = ctx.enter_context(tc.tile_pool(name="xpool", bufs=2))
        sxt = ctx.enter_context(tc.tile_pool(name="sxt", bufs=6))
        hpool = ctx.enter_context(tc.tile_pool(name="hpool", bufs=3))
        opool = ctx.enter_context(tc.tile_pool(name="opool", bufs=3))
        pxt = ctx.enter_context(tc.tile_pool(name="pxt", bufs=3, space="PSUM"))
        zpool = ctx.enter_context(tc.tile_pool(name="zpool", bufs=4, space="PSUM"))

        ident = consts.tile([P, P], bf16)
        make_identity(nc, ident)
        w1_sb = consts.tile([CK, NCH * P], bf16)
        nc.sync.dma_start(w1_sb[:], w1[:])
        w2_sb = consts.tile([P, P], bf16)
        nc.sync.dma_start(w2_sb[:], w2[:])
        w3_sb = consts.tile([P, P], bf16)
        nc.sync.dma_start(w3_sb[:], w3[:])
        wo_sb = consts.tile([P, DO], bf16)
        nc.sync.dma_start(wo_sb[:], wo[:])
        sc = consts.tile([P, 10], f32)
        nc.sync.dma_start(sc[:], scal[:])

        def act_pair(z, h, bias_col, sb_col, s_col):
            # relu channel: h[0:64] = max(z[0:64] + b, 0)
            nc.vector.tensor_scalar(
                h[0:64, :], z[0:64, :],
                sc[0:64, bias_col : bias_col + 1], 0.0,
                mybir.AluOpType.add, mybir.AluOpType.max,
            )
            # tanh channel: h[64:128] = tanh(s*z + s*b)
            nc.scalar.activation(
                h[64:P, :], z[64:P, :],
                mybir.ActivationFunctionType.Tanh,
                bias=sc[64:P, sb_col : sb_col + 1],
                scale=sc[64:P, s_col : s_col + 1],
            )

        def body():
            _body(nc, tc, nblk, xpool, sxt, hpool, opool, pxt, zpool,
                  ident, w1_sb, w2_sb, w3_sb, wo_sb, sc, act_pair, x, out)

        if iters == 1:
            body()
        else:
            # timing mode: repeat the whole (idempotent) kernel body on-device
            with tc.For_i(
                0, iters, 1,
                hint_engines=(
                    mybir.EngineType.PE,
                    mybir.EngineType.DVE,
                    mybir.EngineType.Activation,
                ),
            ):
                body()
    nc.compile()
    return nc


def _body(nc, tc, nblk, xpool, sxt, hpool, opool, pxt, zpool,
          ident, w1_sb, w2_sb, w3_sb, wo_sb, sc, act_pair, x, out):
        f32 = mybir.dt.float32
        bf16 = mybir.dt.bfloat16
        copy_idx = 0
        for blk in range(nblk):
            xt = xpool.tile([P, 8 * D], bf16)
            nc.gpsimd.dma_start(
                xt[:].rearrange("p (n d) -> p n d", n=8),
                x[blk * BLK : (blk + 1) * BLK, :].rearrange(
                    "(n p) d -> p n d", p=P
                ),
            )
            for half in range(2):
                # layer 1: z1[128, 512] accumulated over 7 K-chunks
                z1 = zpool.tile([P, 512], f32, tag="z")
                for cpair in range(4):
                    npairs = 2 if cpair < 3 else 1
                    pt = pxt.tile([CK, 1024], bf16, tag="pt")
                    for ci in range(npairs):
                        c = 2 * cpair + ci
                        for n in range(4):
                            nc.tensor.transpose(
                                pt[:, ci * 512 + n * P : ci * 512 + (n + 1) * P],
                                xt[
                                    :,
                                    (4 * half + n) * D
                                    + c * CK : (4 * half + n) * D
                                    + (c + 1) * CK,
                                ],
                                ident[:],
                            )
                    st = sxt.tile([CK, 1024], bf16, tag="st")
                    for ci in range(npairs):
                        if copy_idx % 2 == 0:
                            nc.vector.tensor_copy(
                                st[:, ci * 512 : (ci + 1) * 512],
                                pt[:, ci * 512 : (ci + 1) * 512],
                            )
                        else:
                            nc.scalar.copy(
                                st[:, ci * 512 : (ci + 1) * 512],
                                pt[:, ci * 512 : (ci + 1) * 512],
                            )
                        copy_idx += 1
                    for ci in range(npairs):
                        c = 2 * cpair + ci
                        nc.tensor.matmul(
                            z1[:],
                            w1_sb[:, c * P : (c + 1) * P],
                            st[:, ci * 512 : (ci + 1) * 512],
                            start=(c == 0),
                            stop=(c == NCH - 1),
                        )
                h1 = hpool.tile([P, 512], bf16, tag="h1")
                act_pair(z1, h1, 0, 1, 2)
                # layer 2
                z2 = zpool.tile([P, 512], f32, tag="z")
                nc.tensor.matmul(z2[:], w2_sb[:], h1[:], start=True, stop=True)
                h2 = hpool.tile([P, 512], bf16, tag="h2")
                act_pair(z2, h2, 3, 4, 5)
                # layer 3
                z3 = zpool.tile([P, 512], f32, tag="z")
                nc.tensor.matmul(z3[:], w3_sb[:], h2[:], start=True, stop=True)
                h3 = hpool.tile([P, 512], bf16, tag="h3")
                act_pair(z3, h3, 6, 7, 8)
                # output layer
                zo = zpool.tile([DO, 512], f32, tag="z")
                nc.tensor.matmul(zo[:], wo_sb[:], h3[:], start=True, stop=True)
                o = opool.tile([DO, 512], f32)
                nc.scalar.activation(
                    o[:], zo[:],
                    mybir.ActivationFunctionType.Identity,
                    bias=sc[0:DO, 9:10], scale=1.0,
                )
                nc.sync.dma_start(
                    out[:, blk * BLK + half * 512 : blk * BLK + (half + 1) * 512],
                    o[:],
                )


def _host_prep(W1, b1, act1, W2, b2, act2, W3, b3, act3, Wo, bo):
    """Fold activation select + biases into weights/scales (tiny, host-side)."""
    f32 = np.float32

    def masks(act):
        m0 = (act == 0).astype(f32)
        m1 = (act == 1).astype(f32)
        m2 = (act == 2).astype(f32)
        q = m1 + 0.5 * m2          # coeff of tanh channel in next layer
        s = 1.0 - 0.5 * m2         # scale inside tanh
        return m0, m2, q, s

    m0_1, m2_1, q1, s1 = masks(act1)
    m0_2, m2_2, q2, s2 = masks(act2)
    m0_3, m2_3, q3, s3 = masks(act3)

    W1 = W1.astype(f32)
    W2 = W2.astype(f32)
    W3 = W3.astype(f32)
    Wo = Wo.astype(f32)

    # layer-1 stationary: chunk c -> [112, 128] with W1_c at cols 0:50 and
    # 64:114 (relu / tanh copies of z1)
    w1p = np.zeros((CK, NCH * P), dtype=BF16)
    for c in range(NCH):
        blk = W1[c * CK : (c + 1) * CK, :].astype(BF16)
        w1p[:, c * P : c * P + D1] = blk
        w1p[:, c * P + 64 : c * P + 64 + D1] = blk

    def fold(W, m0p, qp, double_cols):
        din, dout = W.shape
        ncols = P if double_cols else dout
        o = np.zeros((P, ncols), dtype=f32)
        Wr = W * m0p[:, None]
        Wt = W * qp[:, None]
        o[0:din, 0:dout] = Wr
        o[64 : 64 + din, 0:dout] = Wt
        if double_cols:
            o[0:din, 64 : 64 + dout] = Wr
            o[64 : 64 + din, 64 : 64 + dout] = Wt
        return o.astype(BF16)

    w2p = fold(W2, m0_1, q1, True)    # [128, 128]
    w3p = fold(W3, m0_2, q2, True)    # [128, 128]
    wop = fold(Wo, m0_3, q3, False)   # [128, 10]

    b2p = b2.astype(f32) + 0.5 * (W2.T @ m2_1)
    b3p = b3.astype(f32) + 0.5 * (W3.T @ m2_2)
    bop = bo.astype(f32) + 0.5 * (Wo.T @ m2_3)

    sc = np.zeros((P, 10), dtype=f32)
    sc[0:D1, 0] = b1
    sc[64 : 64 + D1, 1] = s1 * b1
    sc[64 : 64 + D1, 2] = s1
    sc[0:D2, 3] = b2p
    sc[64 : 64 + D2, 4] = s2 * b2p
    sc[64 : 64 + D2, 5] = s2
    sc[0:D3, 6] = b3p
    sc[64 : 64 + D3, 7] = s3 * b3p
    sc[64 : 64 + D3, 8] = s3
    sc[0:DO, 9] = bop
    return w1p, w2p, w3p, wop, sc


last_run = None  # BassKernelResults of the most recent run (for profiling)


def kernel(x, W1, b1, act1, W2, b2, act2, W3, b3, act3, Wo, bo):
    global last_run
    x = np.ascontiguousarray(np.asarray(x, dtype=np.float32))
    x = x.reshape(x.shape[0], -1)
    W1, b1, act1 = np.asarray(W1), np.asarray(b1), np.asarray(act1)
    W2, b2, act2 = np.asarray(W2), np.asarray(b2), np.asarray(act2)
    W3, b3, act3 = np.asarray(W3), np.asarray(b3), np.asarray(act3)
    Wo, bo = np.asarray(Wo), np.asarray(bo)
    nblk = int(os.environ.get("KERNEL_NBLK", NBLK))
    iters = int(os.environ.get("KERNEL_ITERS", "1"))
    rows = nblk * BLK

    w1p, w2p, w3p, wop, sc = _host_prep(
        W1, b1, act1, W2, b2, act2, W3, b3, act3, Wo, bo
    )

    key = (nblk, iters)
    if key not in _prog_cache:
        _prog_cache[key] = _build(nblk, iters)
    nc = _prog_cache[key]

    in_maps = []
    for i in range(NCORES):
        xs = np.ascontiguousarray(x[i * B_CORE : i * B_CORE + rows])
        in_maps.append(
            {"x": xs, "w1": w1p, "w2": w2p, "w3": w3p, "wo": wop, "scal": sc}
        )
    res = run_bass_kernel_spmd(nc, in_maps, core_ids=list(range(NCORES)))
    last_run = res
    out = np.empty((NCORES * rows, DO), dtype=np.float32)
    for i in range(NCORES):
        out[i * rows : (i + 1) * rows] = res.results[i]["out"].T
    return out


# revision 17
# speedup vs baseline: 1.9479x; 1.9479x over previous
"""Trainium2 Bass kernel for nn_EvolvedNetwork_90117003804689 (dense_mlp).

MLP: x[131072,784] -> 50 -> 30 -> 10 -> 10, with per-neuron activation
select (relu/tanh/sigmoid by act code) after each of the 3 hidden layers.

Strategy (pure data parallel over 8 cores, 16384 rows each):
 - Activations are kept TRANSPOSED on chip: hT [neurons, batch], batch on
   the free dim, so layer matmuls are W.T @ hT with tiny stationary weights.
 - x is DMA-loaded with an f32->bf16 cast (SWDGE), transposed 128x112 at a
   time on the PE (identity transpose) into PSUM, copied to SBUF (DVE/ACT
   split) to form the K-major rhs for layer 1.
 - Per-neuron activation select is algebraic, no masks/blends at runtime:
     h = relu_ch + q*tanh_ch + const,  with
     relu channel  r = relu(z + b)                 (DVE tensor_scalar add+max)
     tanh channel  t = tanh(s*(z + b)), s=1-0.5*m2 (ACT, per-partition
                                                    scale/bias)
   sigmoid(v) = 0.5 + 0.5*tanh(v/2) folds into s, q=m1+0.5*m2, and the +0.5
   constant folds into the next layer's bias. The channel masks m0/q fold
   into the next layer's (host-prepared) weights, which take the doubled
   [r; t] stack as input. Each z is produced in doubled form (relu copy at
   partitions 0..d, tanh copy at 64..64+d, zero weight columns elsewhere)
   because engine APs need 32-aligned partition bases and cannot shift
   partitions; the zero columns make PE write exact 0s to gap partitions,
   so downstream reads stay clean.
 - Output is produced as oT [10, 16384] per core; the host transposes and
   concatenates shards.
"""
import os
from contextlib import ExitStack

import numpy as np
import ml_dtypes

from concourse import bacc, mybir
import concourse.tile as tile
from concourse.bass_utils import run_bass_kernel_spmd
from concourse.masks import make_identity

BF16 = ml_dtypes.bfloat16
P = 128
NCORES = 8
B_TOTAL = 131072
B_CORE = B_TOTAL // NCORES  # 16384
D = 784
NCH = 7  # K chunks of 112 for layer 1
CK = 112
D1, D2, D3, DO = 50, 30, 10, 10
BLK = 1024  # rows per block
NBLK = B_CORE // BLK  # 16

# tuning knobs (A/B'd against the cost-model timeline + HW differencing)
DEFAULT_CFG = dict(
    xbufs=3,      # x tile buffering
    stbufs=8,     # SBUF xT chunk tiles
    hbufs=4,      # h tiles per layer tag
    obufs=4,
    ptbufs=2,     # PSUM transpose staging tiles [112,1024] (1 bank each)
    zbufs=3,      # PSUM z tiles [128,1024] f32 (2 banks each)
    merge=True,   # one z[128,1024] per layer per block + pipelined emission
    # engine per xT chunk-copy: cycle through this list
    copy_cycle="v",   # v=vector(DVE), a=scalar(ACT)
    out_eng="v",
    relu_eng="a",     # v=DVE tensor_scalar, a=ACT Relu activation
)

_prog_cache = {}


def _cfg_from_env():
    cfg = dict(DEFAULT_CFG)
    s = os.environ.get("KERNEL_CFG", "")
    if s:
        for kv in s.split(","):
            k, v = kv.split("=")
            if k in ("copy_cycle", "out_eng", "relu_eng"):
                cfg[k] = v
            elif k == "merge":
                cfg[k] = v in ("1", "true")
            else:
                cfg[k] = int(v)
    return cfg


def _build(nblk: int, iters: int = 1, cfg: dict | None = None):
    cfg = dict(DEFAULT_CFG, **(cfg or {}))
    nc = bacc.Bacc("TRN2", target_bir_lowering=False)
    f32 = mybir.dt.float32
    bf16 = mybir.dt.bfloat16

    x = nc.dram_tensor("x", [nblk * BLK, D], f32, kind="ExternalInput")
    w1 = nc.dram_tensor("w1", [CK, NCH * P], bf16, kind="ExternalInput")
    w2 = nc.dram_tensor("w2", [P, P], bf16, kind="ExternalInput")
    w3 = nc.dram_tensor("w3", [P, P], bf16, kind="ExternalInput")
    wo = nc.dram_tensor("wo", [P, DO], bf16, kind="ExternalInput")
    scal = nc.dram_tensor("scal", [P, 10], f32, kind="ExternalInput")
    out = nc.dram_tensor("out", [DO, nblk * BLK], f32, kind="ExternalOutput")

    with ExitStack() as ctx:
        tc = ctx.enter_context(tile.TileContext(nc))
        consts = ctx.enter_context(tc.tile_pool(name="consts", bufs=1))
        xpool = ctx.enter_context(tc.tile_pool(name="xpool", bufs=cfg["xbufs"]))
        sxt = ctx.enter_context(tc.tile_pool(name="sxt", bufs=cfg["stbufs"]))
        hpool = ctx.enter_context(tc.tile_pool(name="hpool", bufs=cfg["hbufs"]))
        opool = ctx.enter_context(tc.tile_pool(name="opool", bufs=cfg["obufs"]))
        pxt = ctx.enter_context(
            tc.tile_pool(name="pxt", bufs=cfg["ptbufs"], space="PSUM")
        )
        zpool = ctx.enter_context(
            tc.tile_pool(name="zpool", bufs=cfg["zbufs"], space="PSUM")
        )

        ident = consts.tile([P, P], bf16)
        make_identity(nc, ident)
        w1_sb = consts.tile([CK, NCH * P], bf16)
        nc.sync.dma_start(w1_sb[:], w1[:])
        w2_sb = consts.tile([P, P], bf16)
        nc.sync.dma_start(w2_sb[:], w2[:])
        w3_sb = consts.tile([P, P], bf16)
        nc.sync.dma_start(w3_sb[:], w3[:])
        wo_sb = consts.tile([P, DO], bf16)
        nc.sync.dma_start(wo_sb[:], wo[:])
        sc = consts.tile([P, 10], f32)
        nc.sync.dma_start(sc[:], scal[:])

        state = {"copy_idx": 0}

        def xt_copy(dst, src):
            cyc = cfg["copy_cycle"]
            eng = cyc[state["copy_idx"] % len(cyc)]
            state["copy_idx"] += 1
            if eng == "v":
                nc.vector.tensor_copy(dst, src)
            else:
                nc.scalar.copy(dst, src)

        def act_pair(z, h, width, bias_col, sb_col, s_col):
            if cfg["relu_eng"] == "v":
                nc.vector.tensor_scalar(
                    h[0:64, 0:width], z[0:64, 0:width],
                    sc[0:64, bias_col : bias_col + 1], 0.0,
                    mybir.AluOpType.add, mybir.AluOpType.max,
                )
            else:
                nc.scalar.activation(
                    h[0:64, 0:width], z[0:64, 0:width],
                    mybir.ActivationFunctionType.Relu,
                    bias=sc[0:64, bias_col : bias_col + 1], scale=1.0,
                )
            nc.scalar.activation(
                h[64:P, 0:width], z[64:P, 0:width],
                mybir.ActivationFunctionType.Tanh,
                bias=sc[64:P, sb_col : sb_col + 1],
                scale=sc[64:P, s_col : s_col + 1],
            )

        def out_pass(o, zo, width):
            if cfg["out_eng"] == "a":
                nc.scalar.activation(
                    o[:, 0:width], zo[:, 0:width],
                    mybir.ActivationFunctionType.Identity,
                    bias=sc[0:DO, 9:10], scale=1.0,
                )
            else:
                nc.vector.tensor_scalar(
                    o[:, 0:width], zo[:, 0:width],
                    sc[0:DO, 9:10], None,
                    mybir.AluOpType.add,
                )

        def body():
            if cfg["merge"]:
                _body_pipelined()
                return
            for blk in range(nblk):
                xt = xpool.tile([P, 8 * D], bf16)
                nc.gpsimd.dma_start(
                    xt[:].rearrange("p (n d) -> p n d", n=8),
                    x[blk * BLK : (blk + 1) * BLK, :].rearrange(
                        "(n p) d -> p n d", p=P
                    ),
                )
                for half in range(2):
                    _half(blk, half, xt)

        def _body_pipelined():
            # software pipeline: block b's transpose/L1 stage is emitted
            # interleaved with block b-1's layer cascade, so the PE always
            # has transpose work queued while ACT/DVE run activation passes
            pipe = {}

            def cascade_step(b, step):
                st = pipe[b]
                if step == 0:
                    h1t = hpool.tile([P, 1024], bf16, tag="h1")
                    st["h1"] = h1t
                    act_pair(st["z1"], h1t, 1024, 0, 1, 2)
                elif step == 1:
                    z2t = zpool.tile([P, 1024], f32, tag="z")
                    st["z2"] = z2t
                    for half in range(2):
                        nc.tensor.matmul(
                            st["z2"][:, half * 512 : (half + 1) * 512],
                            w2_sb[:],
                            st["h1"][:, half * 512 : (half + 1) * 512],
                            start=True, stop=True,
                        )
                elif step == 2:
                    h2t = hpool.tile([P, 1024], bf16, tag="h2")
                    st["h2"] = h2t
                    act_pair(st["z2"], h2t, 1024, 3, 4, 5)
                elif step == 3:
                    z3t = zpool.tile([P, 1024], f32, tag="z")
                    st["z3"] = z3t
                    for half in range(2):
                        nc.tensor.matmul(
                            st["z3"][:, half * 512 : (half + 1) * 512],
                            w3_sb[:],
                            st["h2"][:, half * 512 : (half + 1) * 512],
                            start=True, stop=True,
                        )
                elif step == 4:
                    h3t = hpool.tile([P, 1024], bf16, tag="h3")
                    st["h3"] = h3t
                    act_pair(st["z3"], h3t, 1024, 6, 7, 8)
                elif step == 5:
                    zot = zpool.tile([DO, 1024], f32, tag="z")
                    st["zo"] = zot
                    for half in range(2):
                        nc.tensor.matmul(
                            st["zo"][:, half * 512 : (half + 1) * 512],
                            wo_sb[:],
                            st["h3"][:, half * 512 : (half + 1) * 512],
                            start=True, stop=True,
                        )
                elif step == 6:
                    o = opool.tile([DO, 1024], f32)
                    out_pass(o, st["zo"], 1024)
                    nc.sync.dma_start(
                        out[:, b * BLK : (b + 1) * BLK], o[:]
                    )

            for blk in range(nblk):
                xt = xpool.tile([P, 8 * D], bf16)
                nc.gpsimd.dma_start(
                    xt[:].rearrange("p (n d) -> p n d", n=8),
                    x[blk * BLK : (blk + 1) * BLK, :].rearrange(
                        "(n p) d -> p n d", p=P
                    ),
                )
                z1 = zpool.tile([P, 1024], f32, tag="z")
                pipe[blk] = {"z1": z1}
                sts = {}
                for c in range(NCH + 1):
                    # chunk-level skew: transpose+copy chunk c while the
                    # PE runs chunk c-1's L1 matmuls
                    if c < NCH:
                        pt = pxt.tile([CK, 1024], bf16, tag="pt")
                        _transpose_to(pt, xt, c, range(8), 0)
                        st_t = sxt.tile([CK, 1024], bf16, tag="st")
                        xt_copy(st_t[:], pt[:])
                        sts[c] = st_t
                    if c >= 1:
                        cm = c - 1
                        for half in range(2):
                            nc.tensor.matmul(
                                z1[:, half * 512 : (half + 1) * 512],
                                w1_sb[:, cm * P : (cm + 1) * P],
                                sts[cm][:, half * 512 : (half + 1) * 512],
                                start=(cm == 0),
                                stop=(cm == NCH - 1),
                            )
                        del sts[cm]
                        if blk > 0:
                            cascade_step(blk - 1, cm)
                if blk > 0:
                    del pipe[blk - 1]
            for step in range(NCH):
                cascade_step(nblk - 1, step)
            del pipe[nblk - 1]

        def _transpose_to(pt, xt, c, nlist, col0):
            for k, n in enumerate(nlist):
                nc.tensor.transpose(
                    pt[:, col0 + k * P : col0 + (k + 1) * P],
                    xt[:, n * D + c * CK : n * D + (c + 1) * CK],
                    ident[:],
                )

        def _half(blk, half, xt):
            z1 = zpool.tile([P, 512], f32, tag="z")
            for cpair in range(4):
                npairs = 2 if cpair < 3 else 1
                pt = pxt.tile([CK, 1024], bf16, tag="pt")
                for ci in range(npairs):
                    c = 2 * cpair + ci
                    _transpose_to(
                        pt, xt, c, range(4 * half, 4 * half + 4), ci * 512
                    )
                st = sxt.tile([CK, 1024], bf16, tag="st")
                for ci in range(npairs):
                    xt_copy(
                        st[:, ci * 512 : (ci + 1) * 512],
                        pt[:, ci * 512 : (ci + 1) * 512],
                    )
                for ci in range(npairs):
                    c = 2 * cpair + ci
                    nc.tensor.matmul(
                        z1[:],
                        w1_sb[:, c * P : (c + 1) * P],
                        st[:, ci * 512 : (ci + 1) * 512],
                        start=(c == 0),
                        stop=(c == NCH - 1),
                    )
            h1 = hpool.tile([P, 512], bf16, tag="h1")
            act_pair(z1, h1, 512, 0, 1, 2)
            z2 = zpool.tile([P, 512], f32, tag="z")
            nc.tensor.matmul(z2[:], w2_sb[:], h1[:], start=True, stop=True)
            h2 = hpool.tile([P, 512], bf16, tag="h2")
            act_pair(z2, h2, 512, 3, 4, 5)
            z3 = zpool.tile([P, 512], f32, tag="z")
            nc.tensor.matmul(z3[:], w3_sb[:], h2[:], start=True, stop=True)
            h3 = hpool.tile([P, 512], bf16, tag="h3")
            act_pair(z3, h3, 512, 6, 7, 8)
            zo = zpool.tile([DO, 512], f32, tag="z")
            nc.tensor.matmul(zo[:], wo_sb[:], h3[:], start=True, stop=True)
            o = opool.tile([DO, 512], f32)
            out_pass(o, zo, 512)
            nc.sync.dma_start(
                out[:, blk * BLK + half * 512 : blk * BLK + (half + 1) * 512],
                o[:],
            )

        def _block_merged(blk, xt):
            # one z[128,1024] per layer for the whole 1024-row block
            z1 = zpool.tile([P, 1024], f32, tag="z")
            sts = []
            for c in range(NCH):
                pt = pxt.tile([CK, 1024], bf16, tag="pt")
                _transpose_to(pt, xt, c, range(8), 0)
                st = sxt.tile([CK, 1024], bf16, tag="st")
                xt_copy(st[:], pt[:])
                sts.append(st)
                for half in range(2):
                    nc.tensor.matmul(
                        z1[:, half * 512 : (half + 1) * 512],
                        w1_sb[:, c * P : (c + 1) * P],
                        st[:, half * 512 : (half + 1) * 512],
                        start=(c == 0),
                        stop=(c == NCH - 1),
                    )
            h1 = hpool.tile([P, 1024], bf16, tag="h1")
            act_pair(z1, h1, 1024, 0, 1, 2)
            z2 = zpool.tile([P, 1024], f32, tag="z")
            for half in range(2):
                nc.tensor.matmul(
                    z2[:, half * 512 : (half + 1) * 512],
                    w2_sb[:],
                    h1[:, half * 512 : (half + 1) * 512],
                    start=True, stop=True,
                )
            h2 = hpool.tile([P, 1024], bf16, tag="h2")
            act_pair(z2, h2, 1024, 3, 4, 5)
            z3 = zpool.tile([P, 1024], f32, tag="z")
            for half in range(2):
                nc.tensor.matmul(
                    z3[:, half * 512 : (half + 1) * 512],
                    w3_sb[:],
                    h2[:, half * 512 : (half + 1) * 512],
                    start=True, stop=True,
                )
            h3 = hpool.tile([P, 1024], bf16, tag="h3")
            act_pair(z3, h3, 1024, 6, 7, 8)
            zo = zpool.tile([DO, 1024], f32, tag="z")
            for half in range(2):
                nc.tensor.matmul(
                    zo[:, half * 512 : (half + 1) * 512],
                    wo_sb[:],
                    h3[:, half * 512 : (half + 1) * 512],
                    start=True, stop=True,
                )
            o = opool.tile([DO, 1024], f32)
            out_pass(o, zo, 1024)
            nc.sync.dma_start(out[:, blk * BLK : (blk + 1) * BLK], o[:])

        if iters == 1:
            body()
        else:
            # timing mode: repeat the whole (idempotent) kernel body on-device
            with tc.For_i(
                0, iters, 1,
                hint_engines=(
                    mybir.EngineType.PE,
                    mybir.EngineType.DVE,
                    mybir.EngineType.Activation,
                ),
            ):
                body()
    nc.compile()
    return nc


def _host_prep(W1, b1, act1, W2, b2, act2, W3, b3, act3, Wo, bo):
    """Fold activation select + biases into weights/scales (tiny, host-side)."""
    f32 = np.float32

    def masks(act):
        m0 = (act == 0).astype(f32)
        m1 = (act == 1).astype(f32)
        m2 = (act == 2).astype(f32)
        q = m1 + 0.5 * m2          # coeff of tanh channel in next layer
        s = 1.0 - 0.5 * m2         # scale inside tanh
        return m0, m2, q, s

    m0_1, m2_1, q1, s1 = masks(act1)
    m0_2, m2_2, q2, s2 = masks(act2)
    m0_3, m2_3, q3, s3 = masks(act3)

    W1 = W1.astype(f32)
    W2 = W2.astype(f32)
    W3 = W3.astype(f32)
    Wo = Wo.astype(f32)

    # layer-1 stationary: chunk c -> [112, 128] with W1_c at cols 0:50 and
    # 64:114 (relu / tanh copies of z1)
    w1p = np.zeros((CK, NCH * P), dtype=BF16)
    for c in range(NCH):
        blk = W1[c * CK : (c + 1) * CK, :].astype(BF16)
        w1p[:, c * P : c * P + D1] = blk
        w1p[:, c * P + 64 : c * P + 64 + D1] = blk

    def fold(W, m0p, qp, double_cols):
        din, dout = W.shape
        ncols = P if double_cols else dout
        o = np.zeros((P, ncols), dtype=f32)
        Wr = W * m0p[:, None]
        Wt = W * qp[:, None]
        o[0:din, 0:dout] = Wr
        o[64 : 64 + din, 0:dout] = Wt
        if double_cols:
            o[0:din, 64 : 64 + dout] = Wr
            o[64 : 64 + din, 64 : 64 + dout] = Wt
        return o.astype(BF16)

    w2p = fold(W2, m0_1, q1, True)    # [128, 128]
    w3p = fold(W3, m0_2, q2, True)    # [128, 128]
    wop = fold(Wo, m0_3, q3, False)   # [128, 10]

    b2p = b2.astype(f32) + 0.5 * (W2.T @ m2_1)
    b3p = b3.astype(f32) + 0.5 * (W3.T @ m2_2)
    bop = bo.astype(f32) + 0.5 * (Wo.T @ m2_3)

    sc = np.zeros((P, 10), dtype=f32)
    sc[0:D1, 0] = b1
    sc[64 : 64 + D1, 1] = s1 * b1
    sc[64 : 64 + D1, 2] = s1
    sc[0:D2, 3] = b2p
    sc[64 : 64 + D2, 4] = s2 * b2p
    sc[64 : 64 + D2, 5] = s2
    sc[0:D3, 6] = b3p
    sc[64 : 64 + D3, 7] = s3 * b3p
    sc[64 : 64 + D3, 8] = s3
    sc[0:DO, 9] = bop
    return w1p, w2p, w3p, wop, sc


last_run = None  # BassKernelResults of the most recent run (for profiling)


def kernel(x, W1, b1, act1, W2, b2, act2, W3, b3, act3, Wo, bo):
    global last_run
    x = np.ascontiguousarray(np.asarray(x, dtype=np.float32))
    x = x.reshape(x.shape[0], -1)
    W1, b1, act1 = np.asarray(W1), np.asarray(b1), np.asarray(act1)
    W2, b2, act2 = np.asarray(W2), np.asarray(b2), np.asarray(act2)
    W3, b3, act3 = np.asarray(W3), np.asarray(b3), np.asarray(act3)
    Wo, bo = np.asarray(Wo), np.asarray(bo)
    nblk = int(os.environ.get("KERNEL_NBLK", NBLK))
    iters = int(os.environ.get("KERNEL_ITERS", "1"))
    cfg = _cfg_from_env()
    rows = nblk * BLK

    w1p, w2p, w3p, wop, sc = _host_prep(
        W1, b1, act1, W2, b2, act2, W3, b3, act3, Wo, bo
    )

    key = (nblk, iters, tuple(sorted(cfg.items())))
    if key not in _prog_cache:
        _prog_cache[key] = _build(nblk, iters, cfg)
    nc = _prog_cache[key]

    in_maps = []
    for i in range(NCORES):
        xs = np.ascontiguousarray(x[i * B_CORE : i * B_CORE + rows])
        in_maps.append(
            {"x": xs, "w1": w1p, "w2": w2p, "w3": w3p, "wo": wop, "scal": sc}
        )
    res = run_bass_kernel_spmd(nc, in_maps, core_ids=list(range(NCORES)))
    last_run = res
    out = np.empty((NCORES * rows, DO), dtype=np.float32)
    for i in range(NCORES):
        out[i * rows : (i + 1) * rows] = res.results[i]["out"].T
    return out


# revision 18
# speedup vs baseline: 1.9511x; 1.0017x over previous
"""Trainium2 Bass kernel for nn_EvolvedNetwork_90117003804689 (dense_mlp).

MLP: x[131072,784] -> 50 -> 30 -> 10 -> 10, with per-neuron activation
select (relu/tanh/sigmoid by act code) after each of the 3 hidden layers.

Strategy (pure data parallel over 8 cores, 16384 rows each):
 - Activations are kept TRANSPOSED on chip: hT [neurons, batch], batch on
   the free dim, so layer matmuls are W.T @ hT with tiny stationary weights.
 - x is DMA-loaded with an f32->bf16 cast (SWDGE), transposed 128x112 at a
   time on the PE (identity transpose) into PSUM, copied to SBUF (DVE/ACT
   split) to form the K-major rhs for layer 1.
 - Per-neuron activation select is algebraic, no masks/blends at runtime:
     h = relu_ch + q*tanh_ch + const,  with
     relu channel  r = relu(z + b)                 (DVE tensor_scalar add+max)
     tanh channel  t = tanh(s*(z + b)), s=1-0.5*m2 (ACT, per-partition
                                                    scale/bias)
   sigmoid(v) = 0.5 + 0.5*tanh(v/2) folds into s, q=m1+0.5*m2, and the +0.5
   constant folds into the next layer's bias. The channel masks m0/q fold
   into the next layer's (host-prepared) weights, which take the doubled
   [r; t] stack as input. Each z is produced in doubled form (relu copy at
   partitions 0..d, tanh copy at 64..64+d, zero weight columns elsewhere)
   because engine APs need 32-aligned partition bases and cannot shift
   partitions; the zero columns make PE write exact 0s to gap partitions,
   so downstream reads stay clean.
 - Output is produced as oT [10, 16384] per core; the host transposes and
   concatenates shards.
"""
import os
from contextlib import ExitStack

import numpy as np
import ml_dtypes

from concourse import bacc, mybir
import concourse.tile as tile
from concourse.bass_utils import run_bass_kernel_spmd
from concourse.masks import make_identity

BF16 = ml_dtypes.bfloat16
P = 128
NCORES = 8
B_TOTAL = 131072
B_CORE = B_TOTAL // NCORES  # 16384
D = 784
NCH = 7  # K chunks of 112 for layer 1
CK = 112
D1, D2, D3, DO = 50, 30, 10, 10
BLK = 1024  # rows per block
NBLK = B_CORE // BLK  # 16

# tuning knobs (A/B'd against the cost-model timeline + HW differencing)
DEFAULT_CFG = dict(
    xbufs=3,      # x tile buffering
    stbufs=8,     # SBUF xT chunk tiles
    hbufs=4,      # h tiles per layer tag
    obufs=4,
    ptbufs=2,     # PSUM transpose staging tiles [112,1024] (1 bank each)
    zbufs=3,      # PSUM z tiles [128,1024] f32 (2 banks each)
    merge=True,   # one z[128,1024] per layer per block + pipelined emission
    # engine per xT chunk-copy: cycle through this list
    copy_cycle="v",   # v=vector(DVE), a=scalar(ACT)
    out_eng="v",
    relu_eng="a",     # v=DVE tensor_scalar, a=ACT Relu activation
    nxdma=1,      # x-load DMAs per block
    zotag=0,      # give zo its own psum tag/bufs
    zobufs=1,
)

_prog_cache = {}


def _cfg_from_env():
    cfg = dict(DEFAULT_CFG)
    s = os.environ.get("KERNEL_CFG", "")
    if s:
        for kv in s.split(","):
            k, v = kv.split("=")
            if k in ("copy_cycle", "out_eng", "relu_eng"):
                cfg[k] = v
            elif k == "merge":
                cfg[k] = v in ("1", "true")
            else:
                cfg[k] = int(v)
    return cfg


def _build(nblk: int, iters: int = 1, cfg: dict | None = None):
    cfg = dict(DEFAULT_CFG, **(cfg or {}))
    nc = bacc.Bacc("TRN2", target_bir_lowering=False)
    f32 = mybir.dt.float32
    bf16 = mybir.dt.bfloat16

    x = nc.dram_tensor("x", [nblk * BLK, D], f32, kind="ExternalInput")
    w1 = nc.dram_tensor("w1", [CK, NCH * P], bf16, kind="ExternalInput")
    w2 = nc.dram_tensor("w2", [P, P], bf16, kind="ExternalInput")
    w3 = nc.dram_tensor("w3", [P, P], bf16, kind="ExternalInput")
    wo = nc.dram_tensor("wo", [P, DO], bf16, kind="ExternalInput")
    scal = nc.dram_tensor("scal", [P, 10], f32, kind="ExternalInput")
    out = nc.dram_tensor("out", [DO, nblk * BLK], f32, kind="ExternalOutput")

    with ExitStack() as ctx:
        tc = ctx.enter_context(tile.TileContext(nc))
        consts = ctx.enter_context(tc.tile_pool(name="consts", bufs=1))
        xpool = ctx.enter_context(tc.tile_pool(name="xpool", bufs=cfg["xbufs"]))
        sxt = ctx.enter_context(tc.tile_pool(name="sxt", bufs=cfg["stbufs"]))
        hpool = ctx.enter_context(tc.tile_pool(name="hpool", bufs=cfg["hbufs"]))
        opool = ctx.enter_context(tc.tile_pool(name="opool", bufs=cfg["obufs"]))
        pxt = ctx.enter_context(
            tc.tile_pool(name="pxt", bufs=cfg["ptbufs"], space="PSUM")
        )
        zpool = ctx.enter_context(
            tc.tile_pool(name="zpool", bufs=cfg["zbufs"], space="PSUM")
        )

        ident = consts.tile([P, P], bf16)
        make_identity(nc, ident)
        w1_sb = consts.tile([CK, NCH * P], bf16)
        nc.sync.dma_start(w1_sb[:], w1[:])
        w2_sb = consts.tile([P, P], bf16)
        nc.sync.dma_start(w2_sb[:], w2[:])
        w3_sb = consts.tile([P, P], bf16)
        nc.sync.dma_start(w3_sb[:], w3[:])
        wo_sb = consts.tile([P, DO], bf16)
        nc.sync.dma_start(wo_sb[:], wo[:])
        sc = consts.tile([P, 10], f32)
        nc.sync.dma_start(sc[:], scal[:])

        state = {"copy_idx": 0}

        def xt_copy(dst, src):
            cyc = cfg["copy_cycle"]
            eng = cyc[state["copy_idx"] % len(cyc)]
            state["copy_idx"] += 1
            if eng == "v":
                nc.vector.tensor_copy(dst, src)
            else:
                nc.scalar.copy(dst, src)

        def act_pair(z, h, width, bias_col, sb_col, s_col):
            if cfg["relu_eng"] == "v":
                nc.vector.tensor_scalar(
                    h[0:64, 0:width], z[0:64, 0:width],
                    sc[0:64, bias_col : bias_col + 1], 0.0,
                    mybir.AluOpType.add, mybir.AluOpType.max,
                )
            else:
                nc.scalar.activation(
                    h[0:64, 0:width], z[0:64, 0:width],
                    mybir.ActivationFunctionType.Relu,
                    bias=sc[0:64, bias_col : bias_col + 1], scale=1.0,
                )
            nc.scalar.activation(
                h[64:P, 0:width], z[64:P, 0:width],
                mybir.ActivationFunctionType.Tanh,
                bias=sc[64:P, sb_col : sb_col + 1],
                scale=sc[64:P, s_col : s_col + 1],
            )

        def out_pass(o, zo, width):
            if cfg["out_eng"] == "a":
                nc.scalar.activation(
                    o[:, 0:width], zo[:, 0:width],
                    mybir.ActivationFunctionType.Identity,
                    bias=sc[0:DO, 9:10], scale=1.0,
                )
            else:
                nc.vector.tensor_scalar(
                    o[:, 0:width], zo[:, 0:width],
                    sc[0:DO, 9:10], None,
                    mybir.AluOpType.add,
                )

        def body():
            if cfg["merge"]:
                _body_pipelined()
                return
            for blk in range(nblk):
                xt = xpool.tile([P, 8 * D], bf16)
                nc.gpsimd.dma_start(
                    xt[:].rearrange("p (n d) -> p n d", n=8),
                    x[blk * BLK : (blk + 1) * BLK, :].rearrange(
                        "(n p) d -> p n d", p=P
                    ),
                )
                for half in range(2):
                    _half(blk, half, xt)

        def _body_pipelined():
            # software pipeline: block b's transpose/L1 stage is emitted
            # interleaved with block b-1's layer cascade, so the PE always
            # has transpose work queued while ACT/DVE run activation passes
            pipe = {}

            def cascade_step(b, step):
                st = pipe[b]
                if step == 0:
                    h1t = hpool.tile([P, 1024], bf16, tag="h1")
                    st["h1"] = h1t
                    act_pair(st["z1"], h1t, 1024, 0, 1, 2)
                elif step == 1:
                    z2t = zpool.tile([P, 1024], f32, tag="z")
                    st["z2"] = z2t
                    for half in range(2):
                        nc.tensor.matmul(
                            st["z2"][:, half * 512 : (half + 1) * 512],
                            w2_sb[:],
                            st["h1"][:, half * 512 : (half + 1) * 512],
                            start=True, stop=True,
                        )
                elif step == 2:
                    h2t = hpool.tile([P, 1024], bf16, tag="h2")
                    st["h2"] = h2t
                    act_pair(st["z2"], h2t, 1024, 3, 4, 5)
                elif step == 3:
                    z3t = zpool.tile([P, 1024], f32, tag="z")
                    st["z3"] = z3t
                    for half in range(2):
                        nc.tensor.matmul(
                            st["z3"][:, half * 512 : (half + 1) * 512],
                            w3_sb[:],
                            st["h2"][:, half * 512 : (half + 1) * 512],
                            start=True, stop=True,
                        )
                elif step == 4:
                    h3t = hpool.tile([P, 1024], bf16, tag="h3")
                    st["h3"] = h3t
                    act_pair(st["z3"], h3t, 1024, 6, 7, 8)
                elif step == 5:
                    if cfg["zotag"]:
                        zot = zpool.tile([DO, 1024], f32, tag="zo", bufs=cfg["zobufs"])
                    else:
                        zot = zpool.tile([DO, 1024], f32, tag="z")
                    st["zo"] = zot
                    for half in range(2):
                        nc.tensor.matmul(
                            st["zo"][:, half * 512 : (half + 1) * 512],
                            wo_sb[:],
                            st["h3"][:, half * 512 : (half + 1) * 512],
                            start=True, stop=True,
                        )
                elif step == 6:
                    o = opool.tile([DO, 1024], f32)
                    out_pass(o, st["zo"], 1024)
                    nc.sync.dma_start(
                        out[:, b * BLK : (b + 1) * BLK], o[:]
                    )

            for blk in range(nblk):
                xt = xpool.tile([P, 8 * D], bf16)
                nxd = cfg["nxdma"]
                nsub = 8 // nxd
                for dd in range(nxd):
                    nc.gpsimd.dma_start(
                        xt[:, dd * nsub * D : (dd + 1) * nsub * D].rearrange(
                            "p (n d) -> p n d", n=nsub
                        ),
                        x[
                            blk * BLK + dd * nsub * P : blk * BLK
                            + (dd + 1) * nsub * P,
                            :,
                        ].rearrange("(n p) d -> p n d", p=P),
                    )
                z1 = zpool.tile([P, 1024], f32, tag="z")
                pipe[blk] = {"z1": z1}
                sts = {}
                for c in range(NCH + 1):
                    # chunk-level skew: transpose+copy chunk c while the
                    # PE runs chunk c-1's L1 matmuls
                    if c < NCH:
                        pt = pxt.tile([CK, 1024], bf16, tag="pt")
                        _transpose_to(pt, xt, c, range(8), 0)
                        st_t = sxt.tile([CK, 1024], bf16, tag="st")
                        xt_copy(st_t[:], pt[:])
                        sts[c] = st_t
                    if c >= 1:
                        cm = c - 1
                        for half in range(2):
                            nc.tensor.matmul(
                                z1[:, half * 512 : (half + 1) * 512],
                                w1_sb[:, cm * P : (cm + 1) * P],
                                sts[cm][:, half * 512 : (half + 1) * 512],
                                start=(cm == 0),
                                stop=(cm == NCH - 1),
                            )
                        del sts[cm]
                        if blk > 0:
                            cascade_step(blk - 1, cm)
                if blk > 0:
                    del pipe[blk - 1]
            for step in range(NCH):
                cascade_step(nblk - 1, step)
            del pipe[nblk - 1]

        def _transpose_to(pt, xt, c, nlist, col0):
            for k, n in enumerate(nlist):
                nc.tensor.transpose(
                    pt[:, col0 + k * P : col0 + (k + 1) * P],
                    xt[:, n * D + c * CK : n * D + (c + 1) * CK],
                    ident[:],
                )

        def _half(blk, half, xt):
            z1 = zpool.tile([P, 512], f32, tag="z")
            for cpair in range(4):
                npairs = 2 if cpair < 3 else 1
                pt = pxt.tile([CK, 1024], bf16, tag="pt")
                for ci in range(npairs):
                    c = 2 * cpair + ci
                    _transpose_to(
                        pt, xt, c, range(4 * half, 4 * half + 4), ci * 512
                    )
                st = sxt.tile([CK, 1024], bf16, tag="st")
                for ci in range(npairs):
                    xt_copy(
                        st[:, ci * 512 : (ci + 1) * 512],
                        pt[:, ci * 512 : (ci + 1) * 512],
                    )
                for ci in range(npairs):
                    c = 2 * cpair + ci
                    nc.tensor.matmul(
                        z1[:],
                        w1_sb[:, c * P : (c + 1) * P],
                        st[:, ci * 512 : (ci + 1) * 512],
                        start=(c == 0),
                        stop=(c == NCH - 1),
                    )
            h1 = hpool.tile([P, 512], bf16, tag="h1")
            act_pair(z1, h1, 512, 0, 1, 2)
            z2 = zpool.tile([P, 512], f32, tag="z")
            nc.tensor.matmul(z2[:], w2_sb[:], h1[:], start=True, stop=True)
            h2 = hpool.tile([P, 512], bf16, tag="h2")
            act_pair(z2, h2, 512, 3, 4, 5)
            z3 = zpool.tile([P, 512], f32, tag="z")
            nc.tensor.matmul(z3[:], w3_sb[:], h2[:], start=True, stop=True)
            h3 = hpool.tile([P, 512], bf16, tag="h3")
            act_pair(z3, h3, 512, 6, 7, 8)
            zo = zpool.tile([DO, 512], f32, tag="z")
            nc.tensor.matmul(zo[:], wo_sb[:], h3[:], start=True, stop=True)
            o = opool.tile([DO, 512], f32)
            out_pass(o, zo, 512)
            nc.sync.dma_start(
                out[:, blk * BLK + half * 512 : blk * BLK + (half + 1) * 512],
                o[:],
            )

        def _block_merged(blk, xt):
            # one z[128,1024] per layer for the whole 1024-row block
            z1 = zpool.tile([P, 1024], f32, tag="z")
            sts = []
            for c in range(NCH):
                pt = pxt.tile([CK, 1024], bf16, tag="pt")
                _transpose_to(pt, xt, c, range(8), 0)
                st = sxt.tile([CK, 1024], bf16, tag="st")
                xt_copy(st[:], pt[:])
                sts.append(st)
                for half in range(2):
                    nc.tensor.matmul(
                        z1[:, half * 512 : (half + 1) * 512],
                        w1_sb[:, c * P : (c + 1) * P],
                        st[:, half * 512 : (half + 1) * 512],
                        start=(c == 0),
                        stop=(c == NCH - 1),
                    )
            h1 = hpool.tile([P, 1024], bf16, tag="h1")
            act_pair(z1, h1, 1024, 0, 1, 2)
            z2 = zpool.tile([P, 1024], f32, tag="z")
            for half in range(2):
                nc.tensor.matmul(
                    z2[:, half * 512 : (half + 1) * 512],
                    w2_sb[:],
                    h1[:, half * 512 : (half + 1) * 512],
                    start=True, stop=True,
                )
            h2 = hpool.tile([P, 1024], bf16, tag="h2")
            act_pair(z2, h2, 1024, 3, 4, 5)
            z3 = zpool.tile([P, 1024], f32, tag="z")
            for half in range(2):
                nc.tensor.matmul(
                    z3[:, half * 512 : (half + 1) * 512],
                    w3_sb[:],
                    h2[:, half * 512 : (half + 1) * 512],
                    start=True, stop=True,
                )
            h3 = hpool.tile([P, 1024], bf16, tag="h3")
            act_pair(z3, h3, 1024, 6, 7, 8)
            zo = zpool.tile([DO, 1024], f32, tag="z")
            for half in range(2):
                nc.tensor.matmul(
                    zo[:, half * 512 : (half + 1) * 512],
                    wo_sb[:],
                    h3[:, half * 512 : (half + 1) * 512],
                    start=True, stop=True,
                )
            o = opool.tile([DO, 1024], f32)
            out_pass(o, zo, 1024)
            nc.sync.dma_start(out[:, blk * BLK : (blk + 1) * BLK], o[:])

        if iters == 1:
            body()
        else:
            # timing mode: repeat the whole (idempotent) kernel body on-device
            with tc.For_i(
                0, iters, 1,
                hint_engines=(
                    mybir.EngineType.PE,
                    mybir.EngineType.DVE,
                    mybir.EngineType.Activation,
                ),
            ):
                body()
    nc.compile()
    return nc


def _host_prep(W1, b1, act1, W2, b2, act2, W3, b3, act3, Wo, bo):
    """Fold activation select + biases into weights/scales (tiny, host-side)."""
    f32 = np.float32

    def masks(act):
        m0 = (act == 0).astype(f32)
        m1 = (act == 1).astype(f32)
        m2 = (act == 2).astype(f32)
        q = m1 + 0.5 * m2          # coeff of tanh channel in next layer
        s = 1.0 - 0.5 * m2         # scale inside tanh
        return m0, m2, q, s

    m0_1, m2_1, q1, s1 = masks(act1)
    m0_2, m2_2, q2, s2 = masks(act2)
    m0_3, m2_3, q3, s3 = masks(act3)

    W1 = W1.astype(f32)
    W2 = W2.astype(f32)
    W3 = W3.astype(f32)
    Wo = Wo.astype(f32)

    # layer-1 stationary: chunk c -> [112, 128] with W1_c at cols 0:50 and
    # 64:114 (relu / tanh copies of z1)
    w1p = np.zeros((CK, NCH * P), dtype=BF16)
    for c in range(NCH):
        blk = W1[c * CK : (c + 1) * CK, :].astype(BF16)
        w1p[:, c * P : c * P + D1] = blk
        w1p[:, c * P + 64 : c * P + 64 + D1] = blk

    def fold(W, m0p, qp, double_cols):
        din, dout = W.shape
        ncols = P if double_cols else dout
        o = np.zeros((P, ncols), dtype=f32)
        Wr = W * m0p[:, None]
        Wt = W * qp[:, None]
        o[0:din, 0:dout] = Wr
        o[64 : 64 + din, 0:dout] = Wt
        if double_cols:
            o[0:din, 64 : 64 + dout] = Wr
            o[64 : 64 + din, 64 : 64 + dout] = Wt
        return o.astype(BF16)

    w2p = fold(W2, m0_1, q1, True)    # [128, 128]
    w3p = fold(W3, m0_2, q2, True)    # [128, 128]
    wop = fold(Wo, m0_3, q3, False)   # [128, 10]

    b2p = b2.astype(f32) + 0.5 * (W2.T @ m2_1)
    b3p = b3.astype(f32) + 0.5 * (W3.T @ m2_2)
    bop = bo.astype(f32) + 0.5 * (Wo.T @ m2_3)

    sc = np.zeros((P, 10), dtype=f32)
    sc[0:D1, 0] = b1
    sc[64 : 64 + D1, 1] = s1 * b1
    sc[64 : 64 + D1, 2] = s1
    sc[0:D2, 3] = b2p
    sc[64 : 64 + D2, 4] = s2 * b2p
    sc[64 : 64 + D2, 5] = s2
    sc[0:D3, 6] = b3p
    sc[64 : 64 + D3, 7] = s3 * b3p
    sc[64 : 64 + D3, 8] = s3
    sc[0:DO, 9] = bop
    return w1p, w2p, w3p, wop, sc


last_run = None  # BassKernelResults of the most recent run (for profiling)


def kernel(x, W1, b1, act1, W2, b2, act2, W3, b3, act3, Wo, bo):
    global last_run
    x = np.ascontiguousarray(np.asarray(x, dtype=np.float32))
    x = x.reshape(x.shape[0], -1)
    W1, b1, act1 = np.asarray(W1), np.asarray(b1), np.asarray(act1)
    W2, b2, act2 = np.asarray(W2), np.asarray(b2), np.asarray(act2)
    W3, b3, act3 = np.asarray(W3), np.asarray(b3), np.asarray(act3)
    Wo, bo = np.asarray(Wo), np.asarray(bo)
    nblk = int(os.environ.get("KERNEL_NBLK", NBLK))
    iters = int(os.environ.get("KERNEL_ITERS", "1"))
    cfg = _cfg_from_env()
    rows = nblk * BLK

    w1p, w2p, w3p, wop, sc = _host_prep(
        W1, b1, act1, W2, b2, act2, W3, b3, act3, Wo, bo
    )

    key = (nblk, iters, tuple(sorted(cfg.items())))
    if key not in _prog_cache:
        _prog_cache[key] = _build(nblk, iters, cfg)
    nc = _prog_cache[key]

    in_maps = []
    for i in range(NCORES):
        xs = np.ascontiguousarray(x[i * B_CORE : i * B_CORE + rows])
        in_maps.append(
            {"x": xs, "w1": w1p, "w2": w2p, "w3": w3p, "wo": wop, "scal": sc}
        )
    res = run_bass_kernel_spmd(nc, in_maps, core_ids=list(range(NCORES)))
    last_run = res
    out = np.empty((NCORES * rows, DO), dtype=np.float32)
    for i in range(NCORES):
        out[i * rows : (i + 1) * rows] = res.results[i]["out"].T
    return out
